# revision 13
# baseline (speedup 1.0000x reference)
"""BitLinear (binary group-scaled quantized linear) TRN2 Bass kernel.

y = x @ (sign(w) * s).T + bias, s = max(|scale_group|, 1e-8) per 128-elem
group of flattened w.  Shapes: x [4,2048,4096], w [11008,4096],
bias [11008], scale [352256] -> y [4,2048,11008].

Sharding: column-parallel over out_features across 8 cores (1376 each).
x is replicated; w/scale/bias sliced.  No collectives.

Mixed-precision contraction: the first NF8 k-tiles run as fp8e4
DoubleRow matmuls (2 k-tiles / instruction at 2x PE rate; scale is
pre-rounded to e4m3 on host and folded into the fp8 weights), the
remaining k-tiles run fp16 exactly as the fp16 baseline.  NF8 is chosen
so the quantization error stays under the accuracy budget.
"""

import os
import sys

for _p in ("/opt/trn_rl_repo",):
    if _p not in sys.path and os.path.isdir(_p):
        sys.path.insert(0, _p)

import numpy as np

import concourse.bass as bass
import concourse.mybir as mybir
import concourse.tile as tile
from concourse import bacc
from concourse.bass_utils import run_bass_kernel_spmd

P = 128
N_CORES = 8

# Problem shape (hardcoded per spec nn_BitLinear_65506841199020)
B, S, IN, OUT = 4, 2048, 4096, 11008
T = B * S                      # 8192 rows of x
O_SH = OUT // N_CORES          # 1376 out features per core
K = IN                         # 4096 contraction
KT = K // P                    # 32 k-tiles
GROUP = 128                    # quant group size == P
EPS = 1e-8

NF8 = int(os.environ.get("BITLIN_NF8", "8"))   # fp8 k-tiles (even)
assert NF8 % 2 == 0 and 0 <= NF8 <= KT
# wb8 layout: 0 = [P, 2, o_sh] plane-major; 1 = [P, o_sh, 2] o-major
# (A/B pair bytes adjacent so the PE moving read streams contiguously)
W8ILV = os.environ.get("BITLIN_W8ILV", "0") != "0"

TCH = 256                      # t-columns per x strip chunk
F16 = mybir.dt.float16
BF16 = mybir.dt.bfloat16
F32 = mybir.dt.float32
F8 = mybir.dt.float8e4
DR = mybir.MatmulPerfMode.DoubleRow

LAST_EXEC_NS = None
_NC_CACHE = {}


def _o_blocks(o_sh, blk=512):
    out, o = [], 0
    while o < o_sh:
        w = min(blk, o_sh - o)
        out.append((o, w))
        o += w
    return out


def _emit(nc, tc, tens, t_dim, o_sh, nf8, nf16, tch):
    """Tile kernel body.  tens holds dram handles (see build_nc)."""
    import contextlib

    o_blocks = _o_blocks(o_sh)
    np8 = nf8 // 2
    xT8_r = tens["xT8"][:].rearrange("(kt p) t -> p kt t", p=P) if nf8 else None
    xT16_r = tens["xT16"][:].rearrange("(kt p) t -> p kt t", p=P) if nf16 else None

    # unit = one PSUM-accumulation step: an fp8 pair (2 k-tiles, DoubleRow)
    # or a single fp16 k-tile.  fp16 units first: during the quantize phase
    # the PE is fed per-DMA-byte twice as much by an fp16 tile as by half an
    # fp8 pair, so this ordering minimizes PE starvation while w streams in.
    units = [("f16", i) for i in range(nf16)] + [("f8", j) for j in range(np8)]
    n_units = len(units)

    with contextlib.ExitStack() as ctx:
        const = ctx.enter_context(tc.tile_pool(name="const", bufs=1))
        wload = ctx.enter_context(tc.tile_pool(name="wload", bufs=3))
        sgnp = ctx.enter_context(tc.tile_pool(name="sgn", bufs=3))
        sbc = ctx.enter_context(tc.tile_pool(name="sbc", bufs=3))
        wbinp = ctx.enter_context(tc.tile_pool(name="wbin", bufs=1))
        xs8p = ctx.enter_context(tc.tile_pool(name="xs8", bufs=3))
        xs16p = ctx.enter_context(tc.tile_pool(name="xs16", bufs=3))
        stage = ctx.enter_context(tc.tile_pool(name="stage", bufs=6))
        psum = ctx.enter_context(tc.tile_pool(name="psum", bufs=8, space="PSUM"))

        def load_strip(tci, ndma=8):
            # issued from GpSimd (otherwise idle): keeps the sync engine's
            # DMA queue short — w/scale/y DMAs stay latency-critical on sync
            t0 = tci * tch
            out = []
            for (rearr, nkt, dt, nm) in (
                (xT8_r, nf8, F8, "x8"),
                (xT16_r, nf16, F16, "x16"),
            ):
                if not nkt:
                    out.append(None)
                    continue
                pool = xs8p if nm == "x8" else xs16p
                xs = pool.tile([P, nkt, tch], dt, name=f"{nm}s{tci % 3}",
                               tag=f"{nm}s")
                nd = max(1, min(ndma * nkt // KT, nkt))
                per = (nkt + nd - 1) // nd
                for d in range(0, nkt, per):
                    ke = min(d + per, nkt)
                    nc.gpsimd.dma_start(
                        out=xs[:, d:ke, :], in_=rearr[:, d:ke, t0:t0 + tch]
                    )
                out.append(xs)
            return tuple(out)

        n_ch = t_dim // tch
        n_sub = tch // P
        nblk = len(o_blocks)
        n_rounds = n_ch * n_sub

        # strip 0 queued before the quantize DMAs so the first matmuls can
        # start as soon as the first weights land (queues are FIFO per engine)
        strips = {0: load_strip(0, ndma=16)}
        strip_ndma = 4   # later strips: fewer, larger DMAs (gpsimd issue cost)

        # bias broadcast to all partitions: [P, o_sh] — issued from the
        # tensor engine's queue (idle until the first weights land) so the
        # 704KB broadcast doesn't delay the scale broadcasts on scalar,
        # which gate the first quantized weights and hence the first matmul
        bias_sb = const.tile([P, o_sh], F32)
        nc.gpsimd.dma_start(
            out=bias_sb[:], in_=tens["bias"][:].to_broadcast((P, o_sh))
        )

        def evict_blocks(ps, trow, blocks):
            for bi, (o0, ow) in enumerate(blocks):
                st = stage.tile([P, 512], F32, name=f"st{bi}", tag="st")
                nc.vector.tensor_tensor(
                    out=st[:, :ow], in0=ps[bi][:, :ow],
                    in1=bias_sb[:, o0:o0 + ow], op=mybir.AluOpType.add,
                )
                nc.sync.dma_start(
                    out=tens["y"][trow:trow + P, o0:o0 + ow], in_=st[:, :ow]
                )

        def strip_of(s):
            ch, sub = divmod(s, n_sub)
            return strips[ch], sub

        def mm_unit(u, strip, sub, ps, blocks):
            """Matmuls of unit u into psum tiles ps over o-blocks."""
            kind, idx = units[u]
            xs8_s, xs16_s = strip
            start, stop = u == 0, u == n_units - 1
            if kind == "f8":
                lhsT = xs8_s[:, 2 * idx:2 * idx + 2, sub * P:(sub + 1) * P]
                for bi, (o0, ow) in enumerate(blocks):
                    if W8ILV:
                        rhs = wb8[idx][:, o0:o0 + ow, :].rearrange(
                            "p o two -> p two o"
                        )
                    else:
                        rhs = wb8[idx][:, :, o0:o0 + ow]
                    nc.tensor.matmul(
                        ps[bi][:, :ow], lhsT, rhs,
                        start=start, stop=stop, perf_mode=DR,
                    )
            else:
                lhsT = xs16_s[:, idx, sub * P:(sub + 1) * P]
                for bi, (o0, ow) in enumerate(blocks):
                    nc.tensor.matmul(
                        ps[bi][:, :ow], lhsT, wb16[idx][:, o0:o0 + ow],
                        start=start, stop=stop,
                    )

        # During quantize, PSUM banks cap how much matmul work can overlap.
        # Run NARROW rounds (first 2 o-blocks = 2 banks) for the first 4
        # t-subtiles — 8 banks exactly — so PE consumption tracks weight
        # arrival; the left-over o-block runs densely right after.
        a_blocks = o_blocks[:2] if nblk >= 2 else o_blocks
        b_blocks = o_blocks[len(a_blocks):]
        a_subs = min(4 if nblk >= 2 else 2, n_rounds, 8 // len(a_blocks))
        for c in range(1, (a_subs + n_sub - 1) // n_sub):
            strips[c] = load_strip(c, ndma=strip_ndma)
        fused = [
            [
                psum.tile([P, 512], F32, name=f"fps{s}_{bi}", tag="ps")
                for bi in range(len(a_blocks))
            ]
            for s in range(a_subs)
        ]

        # ---- quantize ----
        # fp8 units: wb8[j][:, i, :] = e4m3(sign(w) * s8), s8 pre-rounded to
        # e4m3 on host so the product (+-1 * s8) casts exactly.
        # fp16 units: wb16[i] = sign(w) * f16(s), as the fp16 baseline.
        wb8_shape = [P, o_sh, 2] if W8ILV else [P, 2, o_sh]
        wb8 = [
            wbinp.tile(wb8_shape, F8, name=f"wb8_{j}", tag=f"wb8_{j}")
            for j in range(np8)
        ]
        wb16 = [None] * nf16
        wsplit = [(i * o_sh) // 4 for i in range(5)]
        n_loaded = 0

        def quant_ktile(src, dst, dst_slice_f8=None):
            """Load w k-tile from dram `src` row, emit sign*scale."""
            nonlocal n_loaded
            wT_t, sT_t, row = src
            wt = wload.tile([P, o_sh], BF16, name="wt", tag="wt")
            if n_loaded < 2:
                # split the first tiles for latency (first MMs gate on them)
                for a, b in zip(wsplit, wsplit[1:]):
                    nc.sync.dma_start(
                        out=wt[:, a:b], in_=wT_t[row * P:(row + 1) * P, a:b]
                    )
            else:
                # split across 2 DMA queues: w arrival gates the quantize
                # chain, which gates PE feed during the startup phase
                h = o_sh // 2
                nc.sync.dma_start(
                    out=wt[:, :h], in_=wT_t[row * P:(row + 1) * P, :h]
                )
                nc.sync.dma_start(
                    out=wt[:, h:], in_=wT_t[row * P:(row + 1) * P, h:]
                )
            sb = sbc.tile([P, o_sh], F16, name="sb", tag="sb")
            # row-DMA (2752B) + on-chip partition broadcast: keeps ~350KB of
            # replicated-scale DMA per tile off the HBM queues, which bound
            # the quantize phase
            sb_eng = nc.scalar if n_loaded < 2 else nc.sync
            sb_eng.dma_start(out=sb[0:1, :], in_=sT_t[row:row + 1, :])
            nc.gpsimd.partition_broadcast(sb[:, :], sb[0:1, :])
            nc.vector.tensor_scalar_max(out=sb[:], in0=sb[:], scalar1=EPS)
            sg = sgnp.tile([P, o_sh], F16, name="sg", tag="sg")
            nc.scalar.activation(
                out=sg[:], in_=wt[:], func=mybir.ActivationFunctionType.Sign
            )
            nc.vector.tensor_mul(out=dst, in0=sg[:], in1=sb[:])
            n_loaded += 1

        for u, (kind, idx) in enumerate(units):
            if kind == "f8":
                for i in range(2):
                    dst = (
                        wb8[idx][:, :, i] if W8ILV else wb8[idx][:, i, :]
                    )
                    quant_ktile(
                        (tens["wT8"], tens["s8T"], 2 * idx + i), dst
                    )
            else:
                wb = wbinp.tile(
                    [P, o_sh], F16, name=f"wb{idx}", tag=f"wbin{idx}"
                )
                quant_ktile((tens["wT16"], tens["s16T"], idx), wb[:])
                wb16[idx] = wb
            for s in range(a_subs):
                strip, sub = strip_of(s)
                mm_unit(u, strip, sub, fused[s], a_blocks)
        for s in range(a_subs):
            _, sub = strip_of(s)
            evict_blocks(fused[s], (s // n_sub) * tch + sub * P, a_blocks)

        # left-over o-range of the startup subtiles: dense full-k rounds
        if b_blocks:
            for s in range(a_subs):
                strip, sub = strip_of(s)
                ps = [
                    psum.tile([P, 512], F32, name=f"bp{bi}", tag="ps")
                    for bi in range(len(b_blocks))
                ]
                for u in range(n_units):
                    mm_unit(u, strip, sub, ps, b_blocks)
                evict_blocks(ps, (s // n_sub) * tch + sub * P, b_blocks)

        # ---- remaining rounds: full o-width, 3 banks each ----
        for s in range(a_subs, n_rounds):
            ch, sub = divmod(s, n_sub)
            if ch not in strips:
                strips[ch] = load_strip(ch, ndma=strip_ndma)
            strip = strips[ch]
            ps = [
                psum.tile([P, 512], F32, name=f"ps{bi}", tag="ps")
                for bi in range(nblk)
            ]
            for u in range(n_units):
                mm_unit(u, strip, sub, ps, o_blocks)
            evict_blocks(ps, ch * tch + sub * P, o_blocks)


def build_nc(t_dim=T, o_sh=O_SH, nf8=NF8, tch=TCH, debug=False):
    key = (t_dim, o_sh, nf8, tch, debug)
    if key in _NC_CACHE:
        return _NC_CACHE[key]
    nf16 = KT - nf8
    nc = bacc.Bacc(
        "TRN2", target_bir_lowering=False, debug=debug, num_devices=N_CORES
    )
    tens = {}
    if nf8:
        tens["xT8"] = nc.dram_tensor(
            "xT8", [nf8 * P, t_dim], F8, kind="ExternalInput"
        )
        tens["wT8"] = nc.dram_tensor(
            "wT8", [nf8 * P, o_sh], BF16, kind="ExternalInput"
        )
        tens["s8T"] = nc.dram_tensor(
            "s8T", [nf8, o_sh], F16, kind="ExternalInput"
        )
    if nf16:
        tens["xT16"] = nc.dram_tensor(
            "xT16", [nf16 * P, t_dim], F16, kind="ExternalInput"
        )
        tens["wT16"] = nc.dram_tensor(
            "wT16", [nf16 * P, o_sh], BF16, kind="ExternalInput"
        )
        tens["s16T"] = nc.dram_tensor(
            "s16T", [nf16, o_sh], F16, kind="ExternalInput"
        )
    tens["bias"] = nc.dram_tensor("bias", [1, o_sh], F32, kind="ExternalInput")
    tens["y"] = nc.dram_tensor("y", [t_dim, o_sh], F32, kind="ExternalOutput")
    with tile.TileContext(nc) as tc:
        _emit(nc, tc, tens, t_dim, o_sh, nf8, nf16, tch)
    nc.compile()
    _NC_CACHE[key] = nc
    return nc


def _prep_inputs(x, weight, bias, scale, nf8=NF8):
    """Host-side sharding/layout prep (dtype/layout only, no math)."""
    import ml_dtypes

    k8 = nf8 * P
    xT = np.ascontiguousarray(x.reshape(T, K).T, dtype=np.float32)  # [K, T]
    xT8 = xT[:k8].astype(ml_dtypes.float8_e4m3) if nf8 else None
    xT16 = xT[k8:].astype(np.float16) if k8 < K else None
    # scale groups: group g of flattened w -> row o = g // (IN//GROUP),
    # k-tile ki = g % (IN//GROUP) since IN % GROUP == 0
    sc = np.maximum(np.abs(scale[: OUT * KT].reshape(OUT, KT)), EPS)
    in_maps = []
    for c in range(N_CORES):
        o0 = c * O_SH
        wTc = np.ascontiguousarray(
            weight[o0:o0 + O_SH, :].T, dtype=np.float32
        )  # [K, O_SH]
        scT = np.ascontiguousarray(sc[o0:o0 + O_SH, :].T, dtype=np.float32)
        m = {
            "bias": np.ascontiguousarray(
                bias[o0:o0 + O_SH], dtype=np.float32
            ).reshape(1, O_SH),
        }
        if nf8:
            m["xT8"] = xT8
            # bf16 cast preserves sign exactly
            m["wT8"] = wTc[:k8].astype(ml_dtypes.bfloat16)
            # pre-round scale to e4m3 so the device-side fold is exact
            m["s8T"] = (
                scT[:nf8].astype(ml_dtypes.float8_e4m3).astype(np.float16)
            )
        if k8 < K:
            m["xT16"] = xT16
            m["wT16"] = wTc[k8:].astype(ml_dtypes.bfloat16)
            m["s16T"] = scT[nf8:].astype(np.float16)
        in_maps.append(m)
    return in_maps


def _install_ntff_hook_shim():
    """The agent image's antenv lacks axon_hooks (a get/set registry), so
    run_bass_kernel_spmd(trace=True) can't find the NTFF profile hook that
    trn_agent_boot would register. Recreate the registry + registration."""
    import types
    import antenv

    if "antenv.axon_hooks" in sys.modules:
        return
    mod = types.ModuleType("antenv.axon_hooks")
    mod._HOOK = None

    def set_axon_ntff_profile_hook(h):
        mod._HOOK = h

    def get_axon_ntff_profile_hook():
        return mod._HOOK

    mod.set_axon_ntff_profile_hook = set_axon_ntff_profile_hook
    mod.get_axon_ntff_profile_hook = get_axon_ntff_profile_hook
    sys.modules["antenv.axon_hooks"] = mod
    antenv.axon_hooks = mod
    try:
        if "/root/.axon_site" not in sys.path and os.path.isdir("/root/.axon_site"):
            sys.path.append("/root/.axon_site")
        from trn_agent_boot.trn_boot import _ntff_profile_via_ctypes

        hook = _ntff_profile_via_ctypes("/opt/axon/libaxon_pjrt.so")
        if hook is not None:
            set_axon_ntff_profile_hook(hook)
    except Exception as e:
        sys.stderr.write(f"ntff hook shim failed: {e!r}\n")


def kernel(x, weight, bias, scale):
    global LAST_EXEC_NS
    nc = build_nc()
    in_maps = _prep_inputs(
        np.asarray(x, dtype=np.float32),
        np.asarray(weight, dtype=np.float32),
        np.asarray(bias, dtype=np.float32),
        np.asarray(scale, dtype=np.float32),
    )
    core_ids = list(range(N_CORES))
    want_trace = os.environ.get("BITLIN_TRACE", "0") != "0"
    res = None
    if want_trace:
        try:
            _install_ntff_hook_shim()
            res = run_bass_kernel_spmd(nc, in_maps, core_ids, trace=True)
            LAST_EXEC_NS = res.exec_time_ns
        except Exception as e:  # fall back to untraced run
            sys.stderr.write(f"kernel: traced run failed ({e!r}); retrying\n")
            res = None
    if res is None:
        res = run_bass_kernel_spmd(nc, in_maps, core_ids)
        LAST_EXEC_NS = res.exec_time_ns
    y = np.concatenate(
        [res.results[c]["y"] for c in range(N_CORES)], axis=1
    )
    return np.ascontiguousarray(y.reshape(B, S, OUT), dtype=np.float32)


# revision 15
# speedup vs baseline: 1.0373x; 1.0373x over previous
"""BitLinear (binary group-scaled quantized linear) TRN2 Bass kernel.

y = x @ (sign(w) * s).T + bias, s = max(|scale_group|, 1e-8) per 128-elem
group of flattened w.  Shapes: x [4,2048,4096], w [11008,4096],
bias [11008], scale [352256] -> y [4,2048,11008].

Sharding: column-parallel over out_features across 8 cores (1376 each).
x is replicated; w/scale/bias sliced.  No collectives.

Mixed-precision contraction: the first NF8 k-tiles run as fp8e4
DoubleRow matmuls (2 k-tiles / instruction at 2x PE rate; scale is
pre-rounded to e4m3 on host and folded into the fp8 weights), the
remaining k-tiles run fp16 exactly as the fp16 baseline.  NF8 is chosen
so the quantization error stays under the accuracy budget.
"""

import os
import sys

for _p in ("/opt/trn_rl_repo",):
    if _p not in sys.path and os.path.isdir(_p):
        sys.path.insert(0, _p)

import numpy as np

import concourse.bass as bass
import concourse.mybir as mybir
import concourse.tile as tile
from concourse import bacc
from concourse.bass_utils import run_bass_kernel_spmd

P = 128
N_CORES = 8

# Problem shape (hardcoded per spec nn_BitLinear_65506841199020)
B, S, IN, OUT = 4, 2048, 4096, 11008
T = B * S                      # 8192 rows of x
O_SH = OUT // N_CORES          # 1376 out features per core
K = IN                         # 4096 contraction
KT = K // P                    # 32 k-tiles
GROUP = 128                    # quant group size == P
EPS = 1e-8

NF8 = int(os.environ.get("BITLIN_NF8", "8"))   # fp8 k-tiles (even)
assert NF8 % 2 == 0 and 0 <= NF8 <= KT
# wb8 layout: 0 = [P, 2, o_sh] plane-major; 1 = [P, o_sh, 2] o-major
# (A/B pair bytes adjacent so the PE moving read streams contiguously)
W8ILV = os.environ.get("BITLIN_W8ILV", "0") != "0"

TCH = 256                      # t-columns per x strip chunk
F16 = mybir.dt.float16
BF16 = mybir.dt.bfloat16
F32 = mybir.dt.float32
F8 = mybir.dt.float8e4
DR = mybir.MatmulPerfMode.DoubleRow

LAST_EXEC_NS = None
_NC_CACHE = {}


def _o_blocks(o_sh, blk=512):
    out, o = [], 0
    while o < o_sh:
        w = min(blk, o_sh - o)
        out.append((o, w))
        o += w
    return out


def _emit(nc, tc, tens, t_dim, o_sh, nf8, nf16, tch):
    """Tile kernel body.  tens holds dram handles (see build_nc)."""
    import contextlib

    o_blocks = _o_blocks(o_sh)
    np8 = nf8 // 2
    xT8_r = tens["xT8"][:].rearrange("(kt p) t -> p kt t", p=P) if nf8 else None
    xT16_r = tens["xT16"][:].rearrange("(kt p) t -> p kt t", p=P) if nf16 else None

    # unit = one PSUM-accumulation step: an fp8 pair (2 k-tiles, DoubleRow)
    # or a single fp16 k-tile.  fp16 units first: during the quantize phase
    # the PE is fed per-DMA-byte twice as much by an fp16 tile as by half an
    # fp8 pair, so this ordering minimizes PE starvation while w streams in.
    units = [("f16", i) for i in range(nf16)] + [("f8", j) for j in range(np8)]
    n_units = len(units)

    with contextlib.ExitStack() as ctx:
        const = ctx.enter_context(tc.tile_pool(name="const", bufs=1))
        wload = ctx.enter_context(tc.tile_pool(name="wload", bufs=3))
        sgnp = ctx.enter_context(tc.tile_pool(name="sgn", bufs=3))
        sbc = ctx.enter_context(tc.tile_pool(name="sbc", bufs=3))
        wbinp = ctx.enter_context(tc.tile_pool(name="wbin", bufs=1))
        xs8p = ctx.enter_context(tc.tile_pool(name="xs8", bufs=3))
        xs16p = ctx.enter_context(tc.tile_pool(name="xs16", bufs=3))
        stage = ctx.enter_context(tc.tile_pool(name="stage", bufs=6))
        psum = ctx.enter_context(tc.tile_pool(name="psum", bufs=8, space="PSUM"))

        def load_strip(tci, ndma=8):
            # issued from GpSimd (otherwise idle): keeps the sync engine's
            # DMA queue short — w/scale/y DMAs stay latency-critical on sync
            t0 = tci * tch
            out = []
            for (rearr, nkt, dt, nm) in (
                (xT8_r, nf8, F8, "x8"),
                (xT16_r, nf16, F16, "x16"),
            ):
                if not nkt:
                    out.append(None)
                    continue
                pool = xs8p if nm == "x8" else xs16p
                xs = pool.tile([P, nkt, tch], dt, name=f"{nm}s{tci % 3}",
                               tag=f"{nm}s")
                nd = max(1, min(ndma * nkt // KT, nkt))
                per = (nkt + nd - 1) // nd
                for d in range(0, nkt, per):
                    ke = min(d + per, nkt)
                    nc.gpsimd.dma_start(
                        out=xs[:, d:ke, :], in_=rearr[:, d:ke, t0:t0 + tch]
                    )
                out.append(xs)
            return tuple(out)

        n_ch = t_dim // tch
        n_sub = tch // P
        nblk = len(o_blocks)
        n_rounds = n_ch * n_sub

        # strip 0 queued before the quantize DMAs so the first matmuls can
        # start as soon as the first weights land (queues are FIFO per engine)
        strips = {0: load_strip(0, ndma=16)}
        strip_ndma = 4   # later strips: fewer, larger DMAs (gpsimd issue cost)

        # bias broadcast to all partitions: [P, o_sh] — issued from the
        # tensor engine's queue (idle until the first weights land) so the
        # 704KB broadcast doesn't delay the scale broadcasts on scalar,
        # which gate the first quantized weights and hence the first matmul
        bias_sb = const.tile([P, o_sh], F32)
        nc.gpsimd.dma_start(
            out=bias_sb[:], in_=tens["bias"][:].to_broadcast((P, o_sh))
        )

        def evict_blocks(ps, trow, blocks):
            for bi, (o0, ow) in enumerate(blocks):
                st = stage.tile([P, 512], F32, name=f"st{bi}", tag="st")
                nc.vector.tensor_tensor(
                    out=st[:, :ow], in0=ps[bi][:, :ow],
                    in1=bias_sb[:, o0:o0 + ow], op=mybir.AluOpType.add,
                )
                nc.sync.dma_start(
                    out=tens["y"][trow:trow + P, o0:o0 + ow], in_=st[:, :ow]
                )

        def strip_of(s):
            ch, sub = divmod(s, n_sub)
            return strips[ch], sub

        def mm_unit(u, strip, sub, ps, blocks):
            """Matmuls of unit u into psum tiles ps over o-blocks."""
            kind, idx = units[u]
            xs8_s, xs16_s = strip
            start, stop = u == 0, u == n_units - 1
            if kind == "f8":
                lhsT = xs8_s[:, 2 * idx:2 * idx + 2, sub * P:(sub + 1) * P]
                for bi, (o0, ow) in enumerate(blocks):
                    if W8ILV:
                        rhs = wb8[idx][:, o0:o0 + ow, :].rearrange(
                            "p o two -> p two o"
                        )
                    else:
                        rhs = wb8[idx][:, :, o0:o0 + ow]
                    nc.tensor.matmul(
                        ps[bi][:, :ow], lhsT, rhs,
                        start=start, stop=stop, perf_mode=DR,
                    )
            else:
                lhsT = xs16_s[:, idx, sub * P:(sub + 1) * P]
                for bi, (o0, ow) in enumerate(blocks):
                    nc.tensor.matmul(
                        ps[bi][:, :ow], lhsT, wb16[idx][:, o0:o0 + ow],
                        start=start, stop=stop,
                    )

        # During quantize, PSUM banks cap how much matmul work can overlap.
        # Run NARROW rounds (first 2 o-blocks = 2 banks) for the first 4
        # t-subtiles — 8 banks exactly — so PE consumption tracks weight
        # arrival; the left-over o-block runs densely right after.
        a_blocks = o_blocks[:2] if nblk >= 2 else o_blocks
        b_blocks = o_blocks[len(a_blocks):]
        a_subs = min(4 if nblk >= 2 else 2, n_rounds, 8 // len(a_blocks))
        for c in range(1, (a_subs + n_sub - 1) // n_sub):
            strips[c] = load_strip(c, ndma=strip_ndma)
        fused = [
            [
                psum.tile([P, 512], F32, name=f"fps{s}_{bi}", tag="ps")
                for bi in range(len(a_blocks))
            ]
            for s in range(a_subs)
        ]

        # ---- quantize ----
        # fp8 units: wb8[j][:, i, :] = e4m3(sign(w) * s8), s8 pre-rounded to
        # e4m3 on host so the product (+-1 * s8) casts exactly.
        # fp16 units: wb16[i] = sign(w) * f16(s), as the fp16 baseline.
        wb8_shape = [P, o_sh, 2] if W8ILV else [P, 2, o_sh]
        wb8 = [
            wbinp.tile(wb8_shape, F8, name=f"wb8_{j}", tag=f"wb8_{j}")
            for j in range(np8)
        ]
        wb16 = [None] * nf16
        wsplit = [(i * o_sh) // 4 for i in range(5)]
        n_loaded = 0

        def quant_ktile(src, dst, dst_slice_f8=None):
            """Load w k-tile from dram `src` row, emit sign*scale."""
            nonlocal n_loaded
            wT_t, sT_t, row = src
            wt = wload.tile([P, o_sh], BF16, name="wt", tag="wt")
            if n_loaded < 2:
                # split the first tiles for latency (first MMs gate on them)
                for a, b in zip(wsplit, wsplit[1:]):
                    nc.sync.dma_start(
                        out=wt[:, a:b], in_=wT_t[row * P:(row + 1) * P, a:b]
                    )
            else:
                # split across 2 DMA queues: w arrival gates the quantize
                # chain, which gates PE feed during the startup phase
                h = o_sh // 2
                nc.sync.dma_start(
                    out=wt[:, :h], in_=wT_t[row * P:(row + 1) * P, :h]
                )
                nc.sync.dma_start(
                    out=wt[:, h:], in_=wT_t[row * P:(row + 1) * P, h:]
                )
            sb = sbc.tile([P, o_sh], F16, name="sb", tag="sb")
            # broadcast reads only 2752B of HBM but writes 352KB through a
            # DMA queue; split across the scalar+sync queues to halve the
            # per-tile write latency (production gates PE feed at startup)
            h = o_sh // 2
            nc.scalar.dma_start(
                out=sb[:, :h], in_=sT_t[row:row + 1, :h].to_broadcast((P, h))
            )
            nc.sync.dma_start(
                out=sb[:, h:],
                in_=sT_t[row:row + 1, h:].to_broadcast((P, o_sh - h)),
            )
            nc.vector.tensor_scalar_max(out=sb[:], in0=sb[:], scalar1=EPS)
            sg = sgnp.tile([P, o_sh], F16, name="sg", tag="sg")
            nc.scalar.activation(
                out=sg[:], in_=wt[:], func=mybir.ActivationFunctionType.Sign
            )
            nc.vector.tensor_mul(out=dst, in0=sg[:], in1=sb[:])
            n_loaded += 1

        for u, (kind, idx) in enumerate(units):
            if kind == "f8":
                for i in range(2):
                    dst = (
                        wb8[idx][:, :, i] if W8ILV else wb8[idx][:, i, :]
                    )
                    quant_ktile(
                        (tens["wT8"], tens["s8T"], 2 * idx + i), dst
                    )
            else:
                wb = wbinp.tile(
                    [P, o_sh], F16, name=f"wb{idx}", tag=f"wbin{idx}"
                )
                quant_ktile((tens["wT16"], tens["s16T"], idx), wb[:])
                wb16[idx] = wb
            for s in range(a_subs):
                strip, sub = strip_of(s)
                mm_unit(u, strip, sub, fused[s], a_blocks)
        for s in range(a_subs):
            _, sub = strip_of(s)
            evict_blocks(fused[s], (s // n_sub) * tch + sub * P, a_blocks)

        # left-over o-range of the startup subtiles: dense full-k rounds
        if b_blocks:
            for s in range(a_subs):
                strip, sub = strip_of(s)
                ps = [
                    psum.tile([P, 512], F32, name=f"bp{bi}", tag="ps")
                    for bi in range(len(b_blocks))
                ]
                for u in range(n_units):
                    mm_unit(u, strip, sub, ps, b_blocks)
                evict_blocks(ps, (s // n_sub) * tch + sub * P, b_blocks)

        # ---- remaining rounds: full o-width, 3 banks each ----
        for s in range(a_subs, n_rounds):
            ch, sub = divmod(s, n_sub)
            if ch not in strips:
                strips[ch] = load_strip(ch, ndma=strip_ndma)
            strip = strips[ch]
            ps = [
                psum.tile([P, 512], F32, name=f"ps{bi}", tag="ps")
                for bi in range(nblk)
            ]
            for u in range(n_units):
                mm_unit(u, strip, sub, ps, o_blocks)
            evict_blocks(ps, ch * tch + sub * P, o_blocks)


def build_nc(t_dim=T, o_sh=O_SH, nf8=NF8, tch=TCH, debug=False):
    key = (t_dim, o_sh, nf8, tch, debug)
    if key in _NC_CACHE:
        return _NC_CACHE[key]
    nf16 = KT - nf8
    nc = bacc.Bacc(
        "TRN2", target_bir_lowering=False, debug=debug, num_devices=N_CORES
    )
    tens = {}
    if nf8:
        tens["xT8"] = nc.dram_tensor(
            "xT8", [nf8 * P, t_dim], F8, kind="ExternalInput"
        )
        tens["wT8"] = nc.dram_tensor(
            "wT8", [nf8 * P, o_sh], BF16, kind="ExternalInput"
        )
        tens["s8T"] = nc.dram_tensor(
            "s8T", [nf8, o_sh], F16, kind="ExternalInput"
        )
    if nf16:
        tens["xT16"] = nc.dram_tensor(
            "xT16", [nf16 * P, t_dim], F16, kind="ExternalInput"
        )
        tens["wT16"] = nc.dram_tensor(
            "wT16", [nf16 * P, o_sh], BF16, kind="ExternalInput"
        )
        tens["s16T"] = nc.dram_tensor(
            "s16T", [nf16, o_sh], F16, kind="ExternalInput"
        )
    tens["bias"] = nc.dram_tensor("bias", [1, o_sh], F32, kind="ExternalInput")
    tens["y"] = nc.dram_tensor("y", [t_dim, o_sh], F32, kind="ExternalOutput")
    with tile.TileContext(nc) as tc:
        _emit(nc, tc, tens, t_dim, o_sh, nf8, nf16, tch)
    nc.compile()
    _NC_CACHE[key] = nc
    return nc


def _prep_inputs(x, weight, bias, scale, nf8=NF8):
    """Host-side sharding/layout prep (dtype/layout only, no math)."""
    import ml_dtypes

    k8 = nf8 * P
    xT = np.ascontiguousarray(x.reshape(T, K).T, dtype=np.float32)  # [K, T]
    xT8 = xT[:k8].astype(ml_dtypes.float8_e4m3) if nf8 else None
    xT16 = xT[k8:].astype(np.float16) if k8 < K else None
    # scale groups: group g of flattened w -> row o = g // (IN//GROUP),
    # k-tile ki = g % (IN//GROUP) since IN % GROUP == 0
    sc = np.maximum(np.abs(scale[: OUT * KT].reshape(OUT, KT)), EPS)
    in_maps = []
    for c in range(N_CORES):
        o0 = c * O_SH
        wTc = np.ascontiguousarray(
            weight[o0:o0 + O_SH, :].T, dtype=np.float32
        )  # [K, O_SH]
        scT = np.ascontiguousarray(sc[o0:o0 + O_SH, :].T, dtype=np.float32)
        m = {
            "bias": np.ascontiguousarray(
                bias[o0:o0 + O_SH], dtype=np.float32
            ).reshape(1, O_SH),
        }
        if nf8:
            m["xT8"] = xT8
            # bf16 cast preserves sign exactly
            m["wT8"] = wTc[:k8].astype(ml_dtypes.bfloat16)
            # pre-round scale to e4m3 so the device-side fold is exact
            m["s8T"] = (
                scT[:nf8].astype(ml_dtypes.float8_e4m3).astype(np.float16)
            )
        if k8 < K:
            m["xT16"] = xT16
            m["wT16"] = wTc[k8:].astype(ml_dtypes.bfloat16)
            m["s16T"] = scT[nf8:].astype(np.float16)
        in_maps.append(m)
    return in_maps


def _install_ntff_hook_shim():
    """The agent image's antenv lacks axon_hooks (a get/set registry), so
    run_bass_kernel_spmd(trace=True) can't find the NTFF profile hook that
    trn_agent_boot would register. Recreate the registry + registration."""
    import types
    import antenv

    if "antenv.axon_hooks" in sys.modules:
        return
    mod = types.ModuleType("antenv.axon_hooks")
    mod._HOOK = None

    def set_axon_ntff_profile_hook(h):
        mod._HOOK = h

    def get_axon_ntff_profile_hook():
        return mod._HOOK

    mod.set_axon_ntff_profile_hook = set_axon_ntff_profile_hook
    mod.get_axon_ntff_profile_hook = get_axon_ntff_profile_hook
    sys.modules["antenv.axon_hooks"] = mod
    antenv.axon_hooks = mod
    try:
        if "/root/.axon_site" not in sys.path and os.path.isdir("/root/.axon_site"):
            sys.path.append("/root/.axon_site")
        from trn_agent_boot.trn_boot import _ntff_profile_via_ctypes

        hook = _ntff_profile_via_ctypes("/opt/axon/libaxon_pjrt.so")
        if hook is not None:
            set_axon_ntff_profile_hook(hook)
    except Exception as e:
        sys.stderr.write(f"ntff hook shim failed: {e!r}\n")


def kernel(x, weight, bias, scale):
    global LAST_EXEC_NS
    nc = build_nc()
    in_maps = _prep_inputs(
        np.asarray(x, dtype=np.float32),
        np.asarray(weight, dtype=np.float32),
        np.asarray(bias, dtype=np.float32),
        np.asarray(scale, dtype=np.float32),
    )
    core_ids = list(range(N_CORES))
    want_trace = os.environ.get("BITLIN_TRACE", "0") != "0"
    res = None
    if want_trace:
        try:
            _install_ntff_hook_shim()
            res = run_bass_kernel_spmd(nc, in_maps, core_ids, trace=True)
            LAST_EXEC_NS = res.exec_time_ns
        except Exception as e:  # fall back to untraced run
            sys.stderr.write(f"kernel: traced run failed ({e!r}); retrying\n")
            res = None
    if res is None:
        res = run_bass_kernel_spmd(nc, in_maps, core_ids)
        LAST_EXEC_NS = res.exec_time_ns
    y = np.concatenate(
        [res.results[c]["y"] for c in range(N_CORES)], axis=1
    )
    return np.ascontiguousarray(y.reshape(B, S, OUT), dtype=np.float32)
